# revision 11
# baseline (speedup 1.0000x reference)
"""AGNNConv on 8 Trainium2 NeuronCores (Bass/Tile).

Math (reference):
    Xp  = X @ W
    e   = beta * <Xp[row], Xp[col]>          per edge
    att = exp(e)
    h_n = sum_{e: row=n} att_e * Xp[col_e]
    s_n = sum_{e: row=n} att_e
    out = h / s

Key algebraic restructuring (W applied zero times per edge):
    e      = <x_col, G x_row>,  G = beta * W W^T   (raw feature space)
    h_raw  = sum att_e * x_col_e                   (raw feature space)
    out    = (h_raw @ W + att_self*m*Xp) / (s + att_self*m)
Self-loop edges (row == col) are peeled off the edge stream and handled
densely per node (att_self = exp(beta*|Xp_n|^2), multiplicity m).

Sharding: destination rows block-partitioned over 8 cores (row is sorted,
so each core owns a contiguous node slice and all edges targeting it --
no cross-core reduction).  Full X is replicated in each core's DRAM as a
kernel input; x_col rows are fetched with dma_gather (int16 indices force
4 column-chunk buckets); G x_row rows are computed locally in phase A and
fetched with a second dma_gather.

Aggregation (race-free, no DMA scatter): edges are sorted by
(512-node window of row, col-chunk); each 128-edge subtile spans <= 64
destination nodes, so a one-hot mask [128 edges, 64 nodes] (built on DVE
from per-edge "row minus subtile window base" values) turns the segment
sum into one PE matmul: H += payload^T @ mask, accumulated in a PSUM
window [65, 512] at a per-subtile dynamic column offset.  One DVE add per
512-node window folds PSUM into a persistent SBUF accumulator h_accT.
payload = [att * x_col | att] so the softmax denominator rides along as
feature 64.
"""

import os
from contextlib import ExitStack
from dataclasses import dataclass

import numpy as np

try:
    from ml_dtypes import bfloat16 as np_bf16
except ImportError:  # pragma: no cover
    np_bf16 = None


# --------------------------------------------------------------------------
# configuration
# --------------------------------------------------------------------------
@dataclass(frozen=True)
class Cfg:
    N: int = 100000        # total nodes
    D: int = 64            # feature dim
    CORES: int = 8
    NCH: int = 4           # column chunks (int16 gather index range)
    SC: int = 12           # subtiles per (window, chunk) group (data-driven)
    WIN: int = 512         # nodes per PSUM accumulation window

    @property
    def NSL_REAL(self):
        return self.N // self.CORES

    @property
    def NSL(self):
        return ((self.NSL_REAL + 1 + 127) // 128) * 128

    @property
    def CHUNK(self):
        c = (self.N + self.NCH - 1) // self.NCH
        c = ((c + 63) // 64) * 64
        assert c <= 32767
        return c

    @property
    def NPAD(self):
        return self.CHUNK * self.NCH

    @property
    def NW(self):
        return (self.NSL + self.WIN - 1) // self.WIN  # windows

    @property
    def WPG(self):
        return 5 if self.NW % 5 == 0 else 1  # windows per gather tile

    @property
    def NGT(self):
        return self.NW // self.WPG           # gather tiles per chunk stream

    @property
    def GTS(self):
        return self.WPG * self.SC * 128      # slots per gather tile

    @property
    def SPS(self):
        return self.NW * self.SC * 128       # slots per chunk stream

    @property
    def NODE_TILES(self):
        return self.NSL // 128


CFG = Cfg()


# --------------------------------------------------------------------------
# device graph
# --------------------------------------------------------------------------
def build_nc(cfg: Cfg):
    from concourse import bacc, mybir, tile
    from concourse.bass import ts, ds

    f32 = mybir.dt.float32
    bf16 = mybir.dt.bfloat16
    i16 = mybir.dt.int16
    i32 = mybir.dt.int32
    D = cfg.D
    Alu = mybir.AluOpType
    Act = mybir.ActivationFunctionType
    GTS = cfg.GTS
    NSUB = cfg.WPG * cfg.SC        # subtiles per gather tile
    HACC_W = cfg.NSL + cfg.WIN     # h_accT column pad so ds(wb,512) stays in range

    nc = bacc.Bacc(
        "TRN2", target_bir_lowering=False, debug=False,
        num_devices=cfg.CORES, num_swdge_queues=4,
    )

    xf = nc.declare_dram_parameter("xf", [cfg.NPAD, D], f32, isOutput=False)
    xsl = nc.declare_dram_parameter("xsl", [cfg.NSL, D], f32, isOutput=False)
    w = nc.declare_dram_parameter("w", [D, D], f32, isOutput=False)
    beta = nc.declare_dram_parameter("beta", [1], f32, isOutput=False)
    colw = nc.declare_dram_parameter(
        "colw", [cfg.NCH * cfg.NGT, 128, GTS // 16], i16, isOutput=False)
    roww = nc.declare_dram_parameter(
        "roww", [cfg.NCH * cfg.NGT, 128, GTS // 16], i16, isOutput=False)
    riw = nc.declare_dram_parameter(
        "riw", [cfg.NCH * cfg.NGT, 128, GTS // 128], bf16, isOutput=False)
    brel = nc.declare_dram_parameter(
        "brel", [1, cfg.NW * cfg.NCH * cfg.SC], i32, isOutput=False)
    mult = nc.declare_dram_parameter("mult", [cfg.NSL, 1], f32, isOutput=False)
    out = nc.declare_dram_parameter("out", [cfg.NSL, D], f32, isOutput=True)

    gx_dram = nc.dram_tensor("gx_scratch", [cfg.NSL, D], f32)

    with ExitStack() as ctx:
        tc = ctx.enter_context(tile.TileContext(nc))
        consts = ctx.enter_context(tc.tile_pool(name="consts", bufs=1))
        keep = ctx.enter_context(tc.tile_pool(name="keep", bufs=1))

        # ---- constants -------------------------------------------------
        ident = consts.tile([128, 128], f32)
        from concourse.masks import make_identity
        make_identity(nc, ident[:])

        w_sb = consts.tile([D, D], f32)
        nc.sync.dma_start(w_sb[:], w[:, :])

        beta64 = consts.tile([D, 1], f32)
        nc.sync.dma_start(beta64[:], beta[:].to_broadcast((D, 1)))
        beta128 = consts.tile([128, 1], f32)
        nc.sync.dma_start(beta128[:], beta[:].to_broadcast((128, 1)))

        # w65: [W | 0; 0 | 1] so one matmul yields [h@W | s]
        w65 = consts.tile([D + 1, D + 1], f32)
        nc.vector.memset(w65[:], 0.0)
        nc.vector.tensor_copy(w65[:D, :D], w_sb[:])
        nc.vector.memset(w65[D:D + 1, D:D + 1], 1.0)

        # iota over the 64-node subtile window, broadcast along subtiles
        iota64 = consts.tile([128, 1, 64], bf16)
        nc.gpsimd.iota(iota64[:], pattern=[[1, 64]], base=0,
                       channel_multiplier=0,
                       allow_small_or_imprecise_dtypes=True)

        zrow1 = consts.tile([1, D + 1], bf16)
        nc.vector.memset(zrow1[:], 0.0)
        zrow512 = consts.tile([1, cfg.WIN], bf16)
        nc.vector.memset(zrow512[:], 0.0)

        brel_sb = consts.tile([1, cfg.NW * cfg.NCH * cfg.SC], i32)
        nc.sync.dma_start(brel_sb[:], brel[:, :])

        # persistent accumulators
        h_accT = keep.tile([D + 1, HACC_W], f32)
        nc.vector.memset(h_accT[:], 0.0)
        xp_all = keep.tile([128, cfg.NODE_TILES, D], f32)
        sattm_all = keep.tile([128, cfg.NODE_TILES, 1], f32)

        # ---- phase A: local node slice ----------------------------------
        with tc.tile_pool(name="sbA", bufs=3) as sb, \
             tc.tile_pool(name="psA", bufs=2, space="PSUM") as ps:

            # G = beta * W @ W^T (symmetric)
            wT_ps = ps.tile([D, D], f32, space="PSUM", tag="pA")
            nc.tensor.transpose(out=wT_ps[:], in_=w_sb[:],
                                identity=ident[:D, :D])
            wT_sb = consts.tile([D, D], f32)
            nc.vector.tensor_copy(wT_sb[:], wT_ps[:])
            g_ps = ps.tile([D, D], f32, space="PSUM", tag="pB")
            nc.tensor.matmul(out=g_ps[:], lhsT=wT_sb[:], rhs=wT_sb[:],
                             start=True, stop=True)
            gb_sb = consts.tile([D, D], f32)
            nc.scalar.mul(gb_sb[:], g_ps[:], beta64[:])

            for t in range(cfg.NODE_TILES):
                xt = sb.tile([128, D], f32, tag="xt")
                nc.sync.dma_start(xt[:], xsl[ts(t, 128), :])
                mult_t = sb.tile([128, 1], f32, tag="mult_t")
                nc.sync.dma_start(mult_t[:], mult[ts(t, 128), :])

                xtT_ps = ps.tile([D, 128], f32, space="PSUM", tag="pA")
                nc.tensor.transpose(out=xtT_ps[:], in_=xt[:],
                                    identity=ident[:])
                xtT_sb = sb.tile([D, 128], f32, tag="xtT_sb")
                nc.any.tensor_copy(xtT_sb[:], xtT_ps[:])

                # (G x)^T -> transpose -> node-major -> DRAM (gather source)
                gx_ps = ps.tile([D, 128], f32, space="PSUM", tag="pB")
                nc.tensor.matmul(out=gx_ps[:], lhsT=gb_sb[:], rhs=xtT_sb[:],
                                 start=True, stop=True)
                gx_sb = sb.tile([D, 128], f32, tag="gx_sb")
                nc.any.tensor_copy(gx_sb[:], gx_ps[:])
                gxT_ps = ps.tile([128, D], f32, space="PSUM", tag="pC")
                nc.tensor.transpose(out=gxT_ps[:], in_=gx_sb[:],
                                    identity=ident[:D, :D])
                gxn = sb.tile([128, D], f32, tag="gxn")
                nc.any.tensor_copy(gxn[:], gxT_ps[:])
                nc.sync.dma_start(gx_dram[ts(t, 128), :], gxn[:])

                # Xp^T -> node-major (kept in SBUF for self path + output)
                xpT_ps = ps.tile([D, 128], f32, space="PSUM", tag="pD")
                nc.tensor.matmul(out=xpT_ps[:], lhsT=w_sb[:], rhs=xtT_sb[:],
                                 start=True, stop=True)
                xpT_sb = sb.tile([D, 128], f32, tag="xpT_sb")
                nc.any.tensor_copy(xpT_sb[:], xpT_ps[:])
                xp_ps = ps.tile([128, D], f32, space="PSUM", tag="pC")
                nc.tensor.transpose(out=xp_ps[:], in_=xpT_sb[:],
                                    identity=ident[:D, :D])
                nc.vector.tensor_copy(xp_all[:, t, :], xp_ps[:])

                # self-loop attention
                sq = sb.tile([128, D], f32, tag="sq")
                nc.vector.tensor_tensor(out=sq[:], in0=xp_all[:, t, :],
                                        in1=xp_all[:, t, :], op=Alu.mult)
                nrm = sb.tile([128, 1], f32, tag="nrm")
                nc.vector.tensor_reduce(out=nrm[:], in_=sq[:],
                                        axis=mybir.AxisListType.X,
                                        op=Alu.add)
                satt = sb.tile([128, 1], f32, tag="satt")
                nc.scalar.activation(out=satt[:], in_=nrm[:], func=Act.Exp,
                                     scale=beta128[:])
                nc.vector.tensor_tensor(out=sattm_all[:, t, :], in0=satt[:],
                                        in1=mult_t[:], op=Alu.mult)

        # ---- phase B: edge stream ---------------------------------------
        with tc.tile_pool(name="sbB", bufs=2) as sbB, \
             tc.tile_pool(name="psB", bufs=1, space="PSUM") as psB:

            for gt in range(cfg.NGT):
                hps = []
                for j in range(cfg.WPG):
                    hps_j = psB.tile([D + 1, cfg.WIN], f32, space="PSUM",
                                     tag=f"hw{j}", name=f"hps_{gt}_{j}")
                    hps.append(hps_j)
                dbg_nozero = bool(int(os.environ.get("AGNN_DBG_NOZERO", "0")))
                if not dbg_nozero:
                    for j in range(cfg.WPG):
                        nc.tensor.matmul(out=hps[j][:], lhsT=zrow1[:],
                                         rhs=zrow512[:], start=True, stop=False,
                                         skip_group_check=True)

                for c in range(cfg.NCH):
                    i = c * cfg.NGT + gt
                    colw_sb = sbB.tile([128, GTS // 16], i16, tag="colw")
                    nc.sync.dma_start(colw_sb[:], colw[i, :, :])
                    roww_sb = sbB.tile([128, GTS // 16], i16, tag="roww")
                    nc.sync.dma_start(roww_sb[:], roww[i, :, :])
                    riw_sb = sbB.tile([128, GTS // 128], bf16, tag="riw")
                    nc.sync.dma_start(riw_sb[:], riw[i, :, :])

                    xq = sbB.tile([128, NSUB, D], f32, tag="xq")
                    nc.gpsimd.dma_gather(
                        xq[:], xf[c * cfg.CHUNK:(c + 1) * cfg.CHUNK, :],
                        colw_sb[:], GTS, GTS, D, single_packet=False)
                    gxr = sbB.tile([128, NSUB, D], f32, tag="gxr")
                    nc.gpsimd.dma_gather(
                        gxr[:], gx_dram[:, :], roww_sb[:], GTS, GTS, D,
                        single_packet=False)

                    # e = <x_col, G x_row>; att = exp(e)  (prod overwrites gxr)
                    nc.vector.tensor_tensor(out=gxr[:], in0=xq[:], in1=gxr[:],
                                            op=Alu.mult)
                    att = sbB.tile([128, NSUB, 1], f32, tag="att")
                    nc.vector.tensor_reduce(out=att[:], in_=gxr[:],
                                            axis=mybir.AxisListType.X,
                                            op=Alu.add)
                    nc.scalar.activation(out=att[:], in_=att[:], func=Act.Exp)

                    # payload = [att * x_col | att]  (bf16)
                    payload = sbB.tile([128, NSUB, D + 1], bf16, tag="payload")
                    nc.vector.tensor_tensor(
                        out=payload[:, :, 0:D], in0=xq[:],
                        in1=att[:].broadcast_to((128, NSUB, D)), op=Alu.mult)
                    nc.scalar.copy(payload[:, :, D:D + 1], att[:])

                    # one-hot mask [128e, 64w] per subtile
                    mask = sbB.tile([128, NSUB, 64], bf16, tag="mask")
                    nc.vector.tensor_tensor(
                        out=mask[:],
                        in0=iota64[:].broadcast_to((128, NSUB, 64)),
                        in1=riw_sb[:, :, None].broadcast_to((128, NSUB, 64)),
                        op=Alu.is_equal)

                    # aggregate: per window, per subtile
                    dbg_static = bool(int(os.environ.get("AGNN_DBG_STATIC", "0")))
                    for j in range(cfg.WPG):
                        woff = ((gt * cfg.WPG + j) * cfg.NCH + c) * cfg.SC
                        if dbg_static:
                            vals = [0] * cfg.SC
                        else:
                            _, vals = nc.values_load_multi_w_load_instructions(
                                brel_sb[0:1, woff:woff + cfg.SC],
                                engines=[mybir.EngineType.PE],
                                min_val=0, max_val=cfg.WIN - 64,
                                skip_runtime_bounds_check=True)
                        for k in range(cfg.SC):
                            s = j * cfg.SC + k
                            last = (c == cfg.NCH - 1) and (k == cfg.SC - 1)
                            nc.tensor.matmul(
                                out=hps[j][:, ds(vals[k], 64)],
                                lhsT=payload[:, s, :], rhs=mask[:, s, :],
                                start=False, stop=last,
                                skip_group_check=True)

                for j in range(cfg.WPG):
                    wb = (gt * cfg.WPG + j) * cfg.WIN
                    nc.vector.tensor_tensor(
                        out=h_accT[:, wb:wb + cfg.WIN],
                        in0=h_accT[:, wb:wb + cfg.WIN],
                        in1=hps[j][:], op=Alu.add)

        # ---- final: combine self path, project, divide -------------------
        with tc.tile_pool(name="sbF", bufs=3) as sbF, \
             tc.tile_pool(name="psF", bufs=2, space="PSUM") as psF:
            for t in range(cfg.NODE_TILES):
                proj_ps = psF.tile([128, D + 1], f32, space="PSUM", tag="pF")
                nc.tensor.matmul(out=proj_ps[:],
                                 lhsT=h_accT[:, ts(t, 128)], rhs=w65[:],
                                 start=True, stop=True)
                hfin = sbF.tile([128, D], f32, tag="hfin")
                nc.vector.scalar_tensor_tensor(
                    out=hfin[:], in0=xp_all[:, t, :], in1=proj_ps[:, 0:D],
                    scalar=sattm_all[:, t, :], op0=Alu.mult, op1=Alu.add)
                s_t = sbF.tile([128, 1], f32, tag="s_t")
                nc.vector.tensor_tensor(out=s_t[:], in0=proj_ps[:, D:D + 1],
                                        in1=sattm_all[:, t, :], op=Alu.add)
                rcp = sbF.tile([128, 1], f32, tag="rcp")
                nc.vector.reciprocal(rcp[:], s_t[:])
                outt = sbF.tile([128, D], f32, tag="outt")
                nc.scalar.mul(outt[:], hfin[:], rcp[:])
                nc.sync.dma_start(out[ts(t, 128), :], outt[:])

    nc.compile()
    return nc


# --------------------------------------------------------------------------
# host-side sharding / index prep
# --------------------------------------------------------------------------
def _pack_group(rows: np.ndarray, wb: int, win: int) -> list:
    """Greedy subtiles: <=128 edges each, rows within [b, b+64),
    b = min(first_row, wb + win - 64). Returns [(start, end, b), ...]."""
    subs = []
    i, n = 0, len(rows)
    while i < n:
        b = min(int(rows[i]), wb + win - 64)
        j = min(i + 128, n)
        j = min(j, int(np.searchsorted(rows, b + 64, side="left")))
        assert j > i
        subs.append((i, j, b))
        i = j
    return subs


def _required_sc(cfg: Cfg, row, col):
    """Max subtiles over all (core, window, chunk) groups."""
    row = np.asarray(row).astype(np.int64)
    col = np.asarray(col).astype(np.int64)
    mx = 0
    for i in range(cfg.CORES):
        lo = i * cfg.NSL_REAL
        sel = (row >= lo) & (row < lo + cfg.NSL_REAL)
        r = row[sel] - lo
        c = col[sel]
        ns = c != r + lo
        r, c = r[ns], c[ns]
        key = (r // cfg.WIN) * cfg.NCH + c // cfg.CHUNK
        order = np.argsort(key, kind="stable")
        r, key = r[order], key[order]
        bounds = np.searchsorted(key, np.arange(cfg.NW * cfg.NCH + 1))
        for g in range(cfg.NW * cfg.NCH):
            a, b = bounds[g], bounds[g + 1]
            if a == b:
                continue
            wb = (g // cfg.NCH) * cfg.WIN
            mx = max(mx, len(_pack_group(r[a:b], wb, cfg.WIN)))
    return mx


def _wrap16(a: np.ndarray) -> np.ndarray:
    """[T, GTS] -> [T, 128, GTS//16]: slot j -> [j%16, j//16], tiled x8."""
    t = a.shape[0]
    w = a.reshape(t, -1, 16).transpose(0, 2, 1)
    return np.ascontiguousarray(np.tile(w, (1, 8, 1)))


def _wrap128(a: np.ndarray) -> np.ndarray:
    """[T, GTS] -> [T, 128, GTS//128]: slot j -> [j%128, j//128]."""
    t = a.shape[0]
    return np.ascontiguousarray(a.reshape(t, -1, 128).transpose(0, 2, 1))


def prep_in_maps(cfg: Cfg, X, W, attention_w, row, col):
    X = np.ascontiguousarray(np.asarray(X, dtype=np.float32))
    W = np.ascontiguousarray(np.asarray(W, dtype=np.float32))
    beta = np.ascontiguousarray(np.asarray(attention_w, dtype=np.float32))
    row = np.asarray(row).astype(np.int64)
    col = np.asarray(col).astype(np.int64)

    xf = np.zeros((cfg.NPAD, cfg.D), dtype=np.float32)
    xf[:cfg.N] = X

    in_maps = []
    for i in range(cfg.CORES):
        lo = i * cfg.NSL_REAL
        hi = lo + cfg.NSL_REAL
        sel = (row >= lo) & (row < hi)
        r = row[sel] - lo
        c = col[sel]

        is_self = (c == r + lo)
        m = np.bincount(r[is_self], minlength=cfg.NSL).astype(np.float32)
        m[cfg.NSL_REAL:] = 1.0

        re = r[~is_self]
        ce = c[~is_self]
        key = (re // cfg.WIN) * cfg.NCH + ce // cfg.CHUNK
        order = np.argsort(key, kind="stable")
        re, ce, key = re[order], ce[order], key[order]
        bounds = np.searchsorted(key, np.arange(cfg.NW * cfg.NCH + 1))

        col16 = np.zeros((cfg.NCH, cfg.SPS), dtype=np.int16)
        row16 = np.zeros((cfg.NCH, cfg.SPS), dtype=np.int16)
        riwf = np.full((cfg.NCH, cfg.SPS), -1.0, dtype=np.float32)
        brel = np.zeros(cfg.NW * cfg.NCH * cfg.SC, dtype=np.int32)

        for wdx in range(cfg.NW):
            wb = wdx * cfg.WIN
            for cc in range(cfg.NCH):
                g = wdx * cfg.NCH + cc
                a, b = bounds[g], bounds[g + 1]
                rg, cg = re[a:b], ce[a:b]
                subs = _pack_group(rg, wb, cfg.WIN) if b > a else []
                assert len(subs) <= cfg.SC, \
                    f"SC overflow: {len(subs)} > {cfg.SC}"
                base = (wdx * cfg.SC) * 128  # slot base in stream cc
                for k, (s0, s1, bsub) in enumerate(subs):
                    sl = base + k * 128
                    n = s1 - s0
                    col16[cc, sl:sl + n] = (cg[s0:s1] - cc * cfg.CHUNK
                                            ).astype(np.int16)
                    row16[cc, sl:sl + n] = rg[s0:s1].astype(np.int16)
                    riwf[cc, sl:sl + n] = (rg[s0:s1] - bsub).astype(np.float32)
                    brel[(wdx * cfg.NCH + cc) * cfg.SC + k] = bsub - wb

        xsl = np.zeros((cfg.NSL, cfg.D), dtype=np.float32)
        xsl[:cfg.NSL_REAL] = X[lo:hi]

        ngt = cfg.NGT
        in_maps.append({
            "xf": xf,
            "xsl": xsl,
            "w": W,
            "beta": beta,
            "colw": _wrap16(col16.reshape(cfg.NCH * ngt, cfg.GTS)),
            "roww": _wrap16(row16.reshape(cfg.NCH * ngt, cfg.GTS)),
            "riw": _wrap128(riwf.reshape(cfg.NCH * ngt, cfg.GTS)
                            ).astype(np_bf16),
            "brel": brel.reshape(1, -1),
            "mult": m.reshape(-1, 1),
        })
    return in_maps


def assemble_out(cfg: Cfg, results) -> np.ndarray:
    parts = [np.asarray(results[i]["out"])[:cfg.NSL_REAL]
             for i in range(cfg.CORES)]
    return np.ascontiguousarray(np.concatenate(parts, axis=0))


# --------------------------------------------------------------------------
# entry point
# --------------------------------------------------------------------------
_NC_CACHE = {}
LAST_RESULT = None


def kernel(X, W, attention_w, row, col) -> np.ndarray:
    global LAST_RESULT
    from concourse.bass_utils import run_bass_kernel_spmd

    sc = _required_sc(CFG, row, col) + 1
    cfg = Cfg(SC=max(sc, CFG.SC))
    if cfg not in _NC_CACHE:
        _NC_CACHE[cfg] = build_nc(cfg)
    nc = _NC_CACHE[cfg]

    in_maps = prep_in_maps(cfg, X, W, attention_w, row, col)
    trace = bool(int(os.environ.get("AGNN_TRACE", "0")))
    res = run_bass_kernel_spmd(
        nc, in_maps, core_ids=list(range(cfg.CORES)), trace=trace)
    LAST_RESULT = res
    return assemble_out(cfg, res.results)


# revision 12
# speedup vs baseline: 1.9386x; 1.9386x over previous
"""AGNNConv on 8 Trainium2 NeuronCores (Bass/Tile).

Math (reference):
    Xp  = X @ W
    e   = beta * <Xp[row], Xp[col]>          per edge
    att = exp(e)
    h_n = sum_{e: row=n} att_e * Xp[col_e]
    s_n = sum_{e: row=n} att_e
    out = h / s

Key algebraic restructuring (W applied zero times per edge):
    e      = <x_col, G x_row>,  G = beta * W W^T   (raw feature space)
    h_raw  = sum att_e * x_col_e                   (raw feature space)
    out    = (h_raw @ W + att_self*m*Xp) / (s + att_self*m)
Self-loop edges (row == col) are peeled off the edge stream and handled
densely per node (att_self = exp(beta*|Xp_n|^2), multiplicity m).

Sharding: destination rows block-partitioned over 8 cores (row is sorted,
so each core owns a contiguous node slice and all edges targeting it --
no cross-core reduction).  Full X is replicated in each core's DRAM as a
kernel input; x_col rows are fetched with dma_gather (int16 indices force
4 column-chunk buckets); G x_row rows are computed locally in phase A and
fetched with a second dma_gather.

Aggregation (race-free, no DMA scatter): edges are sorted by
(512-node window of row, col-chunk); each 128-edge subtile spans <= 64
destination nodes, so a one-hot mask [128 edges, 64 nodes] (built on DVE
from per-edge "row minus subtile window base" values) turns the segment
sum into one PE matmul: H += payload^T @ mask, accumulated in a PSUM
window [65, 512] at a per-subtile dynamic column offset.  One DVE add per
512-node window folds PSUM into a persistent SBUF accumulator h_accT.
payload = [att * x_col | att] so the softmax denominator rides along as
feature 64.
"""

import os
from contextlib import ExitStack
from dataclasses import dataclass

import numpy as np

try:
    from ml_dtypes import bfloat16 as np_bf16
except ImportError:  # pragma: no cover
    np_bf16 = None


# --------------------------------------------------------------------------
# configuration
# --------------------------------------------------------------------------
@dataclass(frozen=True)
class Cfg:
    N: int = 100000        # total nodes
    D: int = 64            # feature dim
    CORES: int = 8
    NCH: int = 4           # column chunks (int16 gather index range)
    SC: int = 12           # subtiles per (window, chunk) group (data-driven)
    WIN: int = 512         # nodes per PSUM accumulation window

    @property
    def NSL_REAL(self):
        return self.N // self.CORES

    @property
    def NSL(self):
        return ((self.NSL_REAL + 1 + 127) // 128) * 128

    @property
    def CHUNK(self):
        c = (self.N + self.NCH - 1) // self.NCH
        c = ((c + 63) // 64) * 64
        assert c <= 32767
        return c

    @property
    def NPAD(self):
        return self.CHUNK * self.NCH

    @property
    def NW(self):
        return (self.NSL + self.WIN - 1) // self.WIN  # windows

    @property
    def WPG(self):
        return 5 if self.NW % 5 == 0 else 1  # windows per gather tile

    @property
    def NGT(self):
        return self.NW // self.WPG           # gather tiles per chunk stream

    @property
    def GTS(self):
        return self.WPG * self.SC * 128      # slots per gather tile

    @property
    def SPS(self):
        return self.NW * self.SC * 128       # slots per chunk stream

    @property
    def NODE_TILES(self):
        return self.NSL // 128


CFG = Cfg()


# --------------------------------------------------------------------------
# device graph
# --------------------------------------------------------------------------
def build_nc(cfg: Cfg):
    from concourse import bacc, mybir, tile
    from concourse.bass import ts, ds

    f32 = mybir.dt.float32
    bf16 = mybir.dt.bfloat16
    i16 = mybir.dt.int16
    i32 = mybir.dt.int32
    D = cfg.D
    Alu = mybir.AluOpType
    Act = mybir.ActivationFunctionType
    GTS = cfg.GTS
    NSUB = cfg.WPG * cfg.SC        # subtiles per gather tile
    HACC_W = cfg.NSL + cfg.WIN     # h_accT column pad so ds(wb,512) stays in range

    nc = bacc.Bacc(
        "TRN2", target_bir_lowering=False, debug=False,
        num_devices=cfg.CORES, num_swdge_queues=4,
    )

    xf = nc.declare_dram_parameter("xf", [cfg.NPAD, D], f32, isOutput=False)
    xsl = nc.declare_dram_parameter("xsl", [cfg.NSL, D], f32, isOutput=False)
    w = nc.declare_dram_parameter("w", [D, D], f32, isOutput=False)
    beta = nc.declare_dram_parameter("beta", [1], f32, isOutput=False)
    colw = nc.declare_dram_parameter(
        "colw", [cfg.NCH * cfg.NGT, 128, GTS // 16], i16, isOutput=False)
    roww = nc.declare_dram_parameter(
        "roww", [cfg.NCH * cfg.NGT, 128, GTS // 16], i16, isOutput=False)
    riw = nc.declare_dram_parameter(
        "riw", [cfg.NCH * cfg.NGT, 128, GTS // 128], bf16, isOutput=False)
    brel = nc.declare_dram_parameter(
        "brel", [1, cfg.NW * cfg.NCH * cfg.SC], i32, isOutput=False)
    mult = nc.declare_dram_parameter("mult", [cfg.NSL, 1], f32, isOutput=False)
    out = nc.declare_dram_parameter("out", [cfg.NSL, D], f32, isOutput=True)

    gx_dram = nc.dram_tensor("gx_scratch", [cfg.NSL, D], f32)

    with ExitStack() as ctx:
        tc = ctx.enter_context(tile.TileContext(nc))
        consts = ctx.enter_context(tc.tile_pool(name="consts", bufs=1))
        keep = ctx.enter_context(tc.tile_pool(name="keep", bufs=1))

        # ---- constants -------------------------------------------------
        ident = consts.tile([128, 128], f32)
        from concourse.masks import make_identity
        make_identity(nc, ident[:])

        w_sb = consts.tile([D, D], f32)
        nc.sync.dma_start(w_sb[:], w[:, :])

        beta64 = consts.tile([D, 1], f32)
        nc.sync.dma_start(beta64[:], beta[:].to_broadcast((D, 1)))
        beta128 = consts.tile([128, 1], f32)
        nc.sync.dma_start(beta128[:], beta[:].to_broadcast((128, 1)))

        # w65: [W | 0; 0 | 1] so one matmul yields [h@W | s]
        w65 = consts.tile([D + 1, D + 1], f32)
        nc.vector.memset(w65[:], 0.0)
        nc.vector.tensor_copy(w65[:D, :D], w_sb[:])
        nc.vector.memset(w65[D:D + 1, D:D + 1], 1.0)

        # iota over the 64-node subtile window, broadcast along subtiles
        iota64 = consts.tile([128, 1, 64], bf16)
        nc.gpsimd.iota(iota64[:], pattern=[[1, 64]], base=0,
                       channel_multiplier=0,
                       allow_small_or_imprecise_dtypes=True)

        zrow1 = consts.tile([1, D + 1], bf16)
        nc.vector.memset(zrow1[:], 0.0)
        zrow512 = consts.tile([1, cfg.WIN], bf16)
        nc.vector.memset(zrow512[:], 0.0)

        brel_sb = consts.tile([1, cfg.NW * cfg.NCH * cfg.SC], i32)
        nc.sync.dma_start(brel_sb[:], brel[:, :])

        # persistent accumulators
        h_accT = keep.tile([D + 1, HACC_W], f32)
        nc.vector.memset(h_accT[:], 0.0)
        xp_all = keep.tile([128, cfg.NODE_TILES, D], f32)
        sattm_all = keep.tile([128, cfg.NODE_TILES, 1], f32)

        # ---- phase A: local node slice ----------------------------------
        with tc.tile_pool(name="sbA", bufs=3) as sb, \
             tc.tile_pool(name="psA", bufs=2, space="PSUM") as ps:

            # G = beta * W @ W^T (symmetric)
            wT_ps = ps.tile([D, D], f32, space="PSUM", tag="pA")
            nc.tensor.transpose(out=wT_ps[:], in_=w_sb[:],
                                identity=ident[:D, :D])
            wT_sb = consts.tile([D, D], f32)
            nc.vector.tensor_copy(wT_sb[:], wT_ps[:])
            g_ps = ps.tile([D, D], f32, space="PSUM", tag="pB")
            nc.tensor.matmul(out=g_ps[:], lhsT=wT_sb[:], rhs=wT_sb[:],
                             start=True, stop=True)
            gb_sb = consts.tile([D, D], f32)
            nc.scalar.mul(gb_sb[:], g_ps[:], beta64[:])

            for t in range(cfg.NODE_TILES):
                xt = sb.tile([128, D], f32, tag="xt")
                nc.sync.dma_start(xt[:], xsl[ts(t, 128), :])
                mult_t = sb.tile([128, 1], f32, tag="mult_t")
                nc.sync.dma_start(mult_t[:], mult[ts(t, 128), :])

                xtT_ps = ps.tile([D, 128], f32, space="PSUM", tag="pA")
                nc.tensor.transpose(out=xtT_ps[:], in_=xt[:],
                                    identity=ident[:])
                xtT_sb = sb.tile([D, 128], f32, tag="xtT_sb")
                nc.any.tensor_copy(xtT_sb[:], xtT_ps[:])

                # (G x)^T -> transpose -> node-major -> DRAM (gather source)
                gx_ps = ps.tile([D, 128], f32, space="PSUM", tag="pB")
                nc.tensor.matmul(out=gx_ps[:], lhsT=gb_sb[:], rhs=xtT_sb[:],
                                 start=True, stop=True)
                gx_sb = sb.tile([D, 128], f32, tag="gx_sb")
                nc.any.tensor_copy(gx_sb[:], gx_ps[:])
                gxT_ps = ps.tile([128, D], f32, space="PSUM", tag="pC")
                nc.tensor.transpose(out=gxT_ps[:], in_=gx_sb[:],
                                    identity=ident[:D, :D])
                gxn = sb.tile([128, D], f32, tag="gxn")
                nc.any.tensor_copy(gxn[:], gxT_ps[:])
                nc.sync.dma_start(gx_dram[ts(t, 128), :], gxn[:])

                # Xp^T -> node-major (kept in SBUF for self path + output)
                xpT_ps = ps.tile([D, 128], f32, space="PSUM", tag="pD")
                nc.tensor.matmul(out=xpT_ps[:], lhsT=w_sb[:], rhs=xtT_sb[:],
                                 start=True, stop=True)
                xpT_sb = sb.tile([D, 128], f32, tag="xpT_sb")
                nc.any.tensor_copy(xpT_sb[:], xpT_ps[:])
                xp_ps = ps.tile([128, D], f32, space="PSUM", tag="pC")
                nc.tensor.transpose(out=xp_ps[:], in_=xpT_sb[:],
                                    identity=ident[:D, :D])
                nc.vector.tensor_copy(xp_all[:, t, :], xp_ps[:])

                # self-loop attention
                sq = sb.tile([128, D], f32, tag="sq")
                nc.vector.tensor_tensor(out=sq[:], in0=xp_all[:, t, :],
                                        in1=xp_all[:, t, :], op=Alu.mult)
                nrm = sb.tile([128, 1], f32, tag="nrm")
                nc.vector.tensor_reduce(out=nrm[:], in_=sq[:],
                                        axis=mybir.AxisListType.X,
                                        op=Alu.add)
                satt = sb.tile([128, 1], f32, tag="satt")
                nc.scalar.activation(out=satt[:], in_=nrm[:], func=Act.Exp,
                                     scale=beta128[:])
                nc.vector.tensor_tensor(out=sattm_all[:, t, :], in0=satt[:],
                                        in1=mult_t[:], op=Alu.mult)

        # ---- phase B: edge stream ---------------------------------------
        with tc.tile_pool(name="sbB", bufs=2) as sbB, \
             tc.tile_pool(name="psB", bufs=1, space="PSUM") as psB:

            for gt in range(cfg.NGT):
                hps = []
                for j in range(cfg.WPG):
                    hps_j = psB.tile([D + 1, cfg.WIN], f32, space="PSUM",
                                     tag=f"hw{j}", name=f"hps_{gt}_{j}")
                    hps.append(hps_j)
                dbg_nozero = bool(int(os.environ.get("AGNN_DBG_NOZERO", "0")))
                if not dbg_nozero:
                    for j in range(cfg.WPG):
                        nc.tensor.matmul(out=hps[j][:], lhsT=zrow1[:],
                                         rhs=zrow512[:], start=True, stop=False,
                                         skip_group_check=True)

                for c in range(cfg.NCH):
                    i = c * cfg.NGT + gt
                    colw_sb = sbB.tile([128, GTS // 16], i16, tag="colw")
                    nc.sync.dma_start(colw_sb[:], colw[i, :, :])
                    roww_sb = sbB.tile([128, GTS // 16], i16, tag="roww")
                    nc.sync.dma_start(roww_sb[:], roww[i, :, :])
                    riw_sb = sbB.tile([128, GTS // 128], bf16, tag="riw")
                    nc.sync.dma_start(riw_sb[:], riw[i, :, :])

                    qi = (c * cfg.NGT + gt) % 2
                    xq = sbB.tile([128, NSUB, D], f32, tag="xq")
                    nc.gpsimd.dma_gather(
                        xq[:], xf[c * cfg.CHUNK:(c + 1) * cfg.CHUNK, :],
                        colw_sb[:], GTS, GTS, D, single_packet=False,
                        queue_num=qi)
                    gxr = sbB.tile([128, NSUB, D], f32, tag="gxr")
                    nc.gpsimd.dma_gather(
                        gxr[:], gx_dram[:, :], roww_sb[:], GTS, GTS, D,
                        single_packet=False, queue_num=2 + qi)

                    # e = <x_col, G x_row>; att = exp(e)  (prod overwrites gxr)
                    nc.vector.tensor_tensor(out=gxr[:], in0=xq[:], in1=gxr[:],
                                            op=Alu.mult)
                    att = sbB.tile([128, NSUB, 1], f32, tag="att")
                    nc.vector.tensor_reduce(out=att[:], in_=gxr[:],
                                            axis=mybir.AxisListType.X,
                                            op=Alu.add)
                    nc.scalar.activation(out=att[:], in_=att[:], func=Act.Exp)

                    # payload = [att * x_col | att]  (bf16)
                    payload = sbB.tile([128, NSUB, D + 1], bf16, tag="payload")
                    nc.vector.tensor_tensor(
                        out=payload[:, :, 0:D], in0=xq[:],
                        in1=att[:].broadcast_to((128, NSUB, D)), op=Alu.mult)
                    nc.scalar.copy(payload[:, :, D:D + 1], att[:])

                    # one-hot mask [128e, 64w] per subtile
                    mask = sbB.tile([128, NSUB, 64], bf16, tag="mask")
                    nc.vector.tensor_tensor(
                        out=mask[:],
                        in0=iota64[:].broadcast_to((128, NSUB, 64)),
                        in1=riw_sb[:, :, None].broadcast_to((128, NSUB, 64)),
                        op=Alu.is_equal)

                    # aggregate: per window, per subtile
                    dbg_static = bool(int(os.environ.get("AGNN_DBG_STATIC", "0")))
                    for j in range(cfg.WPG):
                        woff = ((gt * cfg.WPG + j) * cfg.NCH + c) * cfg.SC
                        if dbg_static:
                            vals = [0] * cfg.SC
                        else:
                            _, vals = nc.values_load_multi_w_load_instructions(
                                brel_sb[0:1, woff:woff + cfg.SC],
                                engines=[mybir.EngineType.PE],
                                min_val=0, max_val=cfg.WIN - 64,
                                skip_runtime_bounds_check=True)
                        for k in range(cfg.SC):
                            s = j * cfg.SC + k
                            last = (c == cfg.NCH - 1) and (k == cfg.SC - 1)
                            nc.tensor.matmul(
                                out=hps[j][:, ds(vals[k], 64)],
                                lhsT=payload[:, s, :], rhs=mask[:, s, :],
                                start=False, stop=last,
                                skip_group_check=True)

                for j in range(cfg.WPG):
                    wb = (gt * cfg.WPG + j) * cfg.WIN
                    nc.vector.tensor_tensor(
                        out=h_accT[:, wb:wb + cfg.WIN],
                        in0=h_accT[:, wb:wb + cfg.WIN],
                        in1=hps[j][:], op=Alu.add)

        # ---- final: combine self path, project, divide -------------------
        with tc.tile_pool(name="sbF", bufs=3) as sbF, \
             tc.tile_pool(name="psF", bufs=2, space="PSUM") as psF:
            for t in range(cfg.NODE_TILES):
                proj_ps = psF.tile([128, D + 1], f32, space="PSUM", tag="pF")
                nc.tensor.matmul(out=proj_ps[:],
                                 lhsT=h_accT[:, ts(t, 128)], rhs=w65[:],
                                 start=True, stop=True)
                hfin = sbF.tile([128, D], f32, tag="hfin")
                nc.vector.scalar_tensor_tensor(
                    out=hfin[:], in0=xp_all[:, t, :], in1=proj_ps[:, 0:D],
                    scalar=sattm_all[:, t, :], op0=Alu.mult, op1=Alu.add)
                s_t = sbF.tile([128, 1], f32, tag="s_t")
                nc.vector.tensor_tensor(out=s_t[:], in0=proj_ps[:, D:D + 1],
                                        in1=sattm_all[:, t, :], op=Alu.add)
                rcp = sbF.tile([128, 1], f32, tag="rcp")
                nc.vector.reciprocal(rcp[:], s_t[:])
                outt = sbF.tile([128, D], f32, tag="outt")
                nc.scalar.mul(outt[:], hfin[:], rcp[:])
                nc.sync.dma_start(out[ts(t, 128), :], outt[:])

    nc.compile()
    return nc


# --------------------------------------------------------------------------
# host-side sharding / index prep
# --------------------------------------------------------------------------
def _pack_group(rows: np.ndarray, wb: int, win: int) -> list:
    """Greedy subtiles: <=128 edges each, rows within [b, b+64),
    b = min(first_row, wb + win - 64). Returns [(start, end, b), ...]."""
    subs = []
    i, n = 0, len(rows)
    while i < n:
        b = min(int(rows[i]), wb + win - 64)
        j = min(i + 128, n)
        j = min(j, int(np.searchsorted(rows, b + 64, side="left")))
        assert j > i
        subs.append((i, j, b))
        i = j
    return subs


def _required_sc(cfg: Cfg, row, col):
    """Max subtiles over all (core, window, chunk) groups."""
    row = np.asarray(row).astype(np.int64)
    col = np.asarray(col).astype(np.int64)
    mx = 0
    for i in range(cfg.CORES):
        lo = i * cfg.NSL_REAL
        sel = (row >= lo) & (row < lo + cfg.NSL_REAL)
        r = row[sel] - lo
        c = col[sel]
        ns = c != r + lo
        r, c = r[ns], c[ns]
        key = (r // cfg.WIN) * cfg.NCH + c // cfg.CHUNK
        order = np.argsort(key, kind="stable")
        r, key = r[order], key[order]
        bounds = np.searchsorted(key, np.arange(cfg.NW * cfg.NCH + 1))
        for g in range(cfg.NW * cfg.NCH):
            a, b = bounds[g], bounds[g + 1]
            if a == b:
                continue
            wb = (g // cfg.NCH) * cfg.WIN
            mx = max(mx, len(_pack_group(r[a:b], wb, cfg.WIN)))
    return mx


def _wrap16(a: np.ndarray) -> np.ndarray:
    """[T, GTS] -> [T, 128, GTS//16]: slot j -> [j%16, j//16], tiled x8."""
    t = a.shape[0]
    w = a.reshape(t, -1, 16).transpose(0, 2, 1)
    return np.ascontiguousarray(np.tile(w, (1, 8, 1)))


def _wrap128(a: np.ndarray) -> np.ndarray:
    """[T, GTS] -> [T, 128, GTS//128]: slot j -> [j%128, j//128]."""
    t = a.shape[0]
    return np.ascontiguousarray(a.reshape(t, -1, 128).transpose(0, 2, 1))


def prep_in_maps(cfg: Cfg, X, W, attention_w, row, col):
    X = np.ascontiguousarray(np.asarray(X, dtype=np.float32))
    W = np.ascontiguousarray(np.asarray(W, dtype=np.float32))
    beta = np.ascontiguousarray(np.asarray(attention_w, dtype=np.float32))
    row = np.asarray(row).astype(np.int64)
    col = np.asarray(col).astype(np.int64)

    xf = np.zeros((cfg.NPAD, cfg.D), dtype=np.float32)
    xf[:cfg.N] = X

    in_maps = []
    for i in range(cfg.CORES):
        lo = i * cfg.NSL_REAL
        hi = lo + cfg.NSL_REAL
        sel = (row >= lo) & (row < hi)
        r = row[sel] - lo
        c = col[sel]

        is_self = (c == r + lo)
        m = np.bincount(r[is_self], minlength=cfg.NSL).astype(np.float32)
        m[cfg.NSL_REAL:] = 1.0

        re = r[~is_self]
        ce = c[~is_self]
        key = (re // cfg.WIN) * cfg.NCH + ce // cfg.CHUNK
        order = np.argsort(key, kind="stable")
        re, ce, key = re[order], ce[order], key[order]
        bounds = np.searchsorted(key, np.arange(cfg.NW * cfg.NCH + 1))

        col16 = np.zeros((cfg.NCH, cfg.SPS), dtype=np.int16)
        row16 = np.zeros((cfg.NCH, cfg.SPS), dtype=np.int16)
        riwf = np.full((cfg.NCH, cfg.SPS), -1.0, dtype=np.float32)
        brel = np.zeros(cfg.NW * cfg.NCH * cfg.SC, dtype=np.int32)

        for wdx in range(cfg.NW):
            wb = wdx * cfg.WIN
            for cc in range(cfg.NCH):
                g = wdx * cfg.NCH + cc
                a, b = bounds[g], bounds[g + 1]
                rg, cg = re[a:b], ce[a:b]
                subs = _pack_group(rg, wb, cfg.WIN) if b > a else []
                assert len(subs) <= cfg.SC, \
                    f"SC overflow: {len(subs)} > {cfg.SC}"
                base = (wdx * cfg.SC) * 128  # slot base in stream cc
                for k, (s0, s1, bsub) in enumerate(subs):
                    sl = base + k * 128
                    n = s1 - s0
                    col16[cc, sl:sl + n] = (cg[s0:s1] - cc * cfg.CHUNK
                                            ).astype(np.int16)
                    row16[cc, sl:sl + n] = rg[s0:s1].astype(np.int16)
                    riwf[cc, sl:sl + n] = (rg[s0:s1] - bsub).astype(np.float32)
                    brel[(wdx * cfg.NCH + cc) * cfg.SC + k] = bsub - wb

        xsl = np.zeros((cfg.NSL, cfg.D), dtype=np.float32)
        xsl[:cfg.NSL_REAL] = X[lo:hi]

        ngt = cfg.NGT
        in_maps.append({
            "xf": xf,
            "xsl": xsl,
            "w": W,
            "beta": beta,
            "colw": _wrap16(col16.reshape(cfg.NCH * ngt, cfg.GTS)),
            "roww": _wrap16(row16.reshape(cfg.NCH * ngt, cfg.GTS)),
            "riw": _wrap128(riwf.reshape(cfg.NCH * ngt, cfg.GTS)
                            ).astype(np_bf16),
            "brel": brel.reshape(1, -1),
            "mult": m.reshape(-1, 1),
        })
    return in_maps


def assemble_out(cfg: Cfg, results) -> np.ndarray:
    parts = [np.asarray(results[i]["out"])[:cfg.NSL_REAL]
             for i in range(cfg.CORES)]
    return np.ascontiguousarray(np.concatenate(parts, axis=0))


# --------------------------------------------------------------------------
# entry point
# --------------------------------------------------------------------------
_NC_CACHE = {}
LAST_RESULT = None


def kernel(X, W, attention_w, row, col) -> np.ndarray:
    global LAST_RESULT
    from concourse.bass_utils import run_bass_kernel_spmd

    sc = _required_sc(CFG, row, col) + 1
    cfg = Cfg(SC=max(sc, CFG.SC))
    if cfg not in _NC_CACHE:
        _NC_CACHE[cfg] = build_nc(cfg)
    nc = _NC_CACHE[cfg]

    in_maps = prep_in_maps(cfg, X, W, attention_w, row, col)
    trace = bool(int(os.environ.get("AGNN_TRACE", "0")))
    res = run_bass_kernel_spmd(
        nc, in_maps, core_ids=list(range(cfg.CORES)), trace=trace)
    LAST_RESULT = res
    return assemble_out(cfg, res.results)


# revision 16
# speedup vs baseline: 3.5065x; 1.8088x over previous
"""AGNNConv on 8 Trainium2 NeuronCores (Bass/Tile).

Math (reference):
    Xp  = X @ W
    e   = beta * <Xp[row], Xp[col]>          per edge
    att = exp(e)
    h_n = sum_{e: row=n} att_e * Xp[col_e]
    s_n = sum_{e: row=n} att_e
    out = h / s

Key algebraic restructuring (W applied zero times per edge):
    e      = <x_col, G x_row>,  G = beta * W W^T   (raw feature space)
    h_raw  = sum att_e * x_col_e                   (raw feature space)
    out    = (h_raw @ W + att_self*m*Xp) / (s + att_self*m)
Self-loop edges (row == col) are peeled off the edge stream and handled
densely per node (att_self = exp(beta*|Xp_n|^2), multiplicity m).

Sharding: destination rows block-partitioned over 8 cores (row is sorted,
so each core owns a contiguous node slice and all edges targeting it --
no cross-core reduction).  Full X is replicated in each core's DRAM as a
kernel input; x_col rows are fetched with dma_gather (int16 indices force
4 column-chunk buckets); G x_row rows are computed locally in phase A and
fetched with a second dma_gather.

Aggregation (race-free, no DMA scatter): edges are sorted by
(512-node window of row, col-chunk); each 128-edge subtile spans <= 64
destination nodes, so a one-hot mask [128 edges, 64 nodes] (built on DVE
from per-edge "row minus subtile window base" values) turns the segment
sum into one PE matmul: H += payload^T @ mask, accumulated in a PSUM
window [65, 512] at a per-subtile dynamic column offset.  One DVE add per
512-node window folds PSUM into a persistent SBUF accumulator h_accT.
payload = [att * x_col | att] so the softmax denominator rides along as
feature 64.
"""

import os
from contextlib import ExitStack
from dataclasses import dataclass

import numpy as np

try:
    from ml_dtypes import bfloat16 as np_bf16
except ImportError:  # pragma: no cover
    np_bf16 = None


# --------------------------------------------------------------------------
# configuration
# --------------------------------------------------------------------------
@dataclass(frozen=True)
class Cfg:
    N: int = 100000        # total nodes
    D: int = 64            # feature dim
    CORES: int = 8
    NCH: int = 4           # column chunks (int16 gather index range)
    SC: int = 12           # subtiles per (window, chunk) group (data-driven)
    WIN: int = 512         # nodes per PSUM accumulation window

    @property
    def NSL_REAL(self):
        return self.N // self.CORES

    @property
    def NSL(self):
        return ((self.NSL_REAL + 1 + 127) // 128) * 128

    @property
    def CHUNK(self):
        c = (self.N + self.NCH - 1) // self.NCH
        c = ((c + 63) // 64) * 64
        assert c <= 32767
        return c

    @property
    def NPAD(self):
        return self.CHUNK * self.NCH

    @property
    def NW(self):
        return (self.NSL + self.WIN - 1) // self.WIN  # windows

    @property
    def WPG(self):
        return 5 if self.NW % 5 == 0 else 1  # windows per gather tile

    @property
    def NGT(self):
        return self.NW // self.WPG           # gather tiles per chunk stream

    @property
    def GTS(self):
        return self.WPG * self.SC * 128      # slots per gather tile

    @property
    def SPS(self):
        return self.NW * self.SC * 128       # slots per chunk stream

    @property
    def NODE_TILES(self):
        return self.NSL // 128


CFG = Cfg()


# --------------------------------------------------------------------------
# device graph
# --------------------------------------------------------------------------
def build_nc(cfg: Cfg):
    from concourse import bacc, mybir, tile
    from concourse.bass import ts, ds

    f32 = mybir.dt.float32
    bf16 = mybir.dt.bfloat16
    i16 = mybir.dt.int16
    i32 = mybir.dt.int32
    D = cfg.D
    Alu = mybir.AluOpType
    Act = mybir.ActivationFunctionType
    GTS = cfg.GTS
    NSUB = cfg.WPG * cfg.SC        # subtiles per gather tile
    HACC_W = cfg.NSL + cfg.WIN     # h_accT column pad so ds(wb,512) stays in range

    nc = bacc.Bacc(
        "TRN2", target_bir_lowering=False, debug=False,
        num_devices=cfg.CORES, num_swdge_queues=4,
    )

    xf = nc.declare_dram_parameter("xf", [cfg.NPAD, D], f32, isOutput=False)
    xsl = nc.declare_dram_parameter("xsl", [cfg.NSL, D], f32, isOutput=False)
    w = nc.declare_dram_parameter("w", [D, D], f32, isOutput=False)
    beta = nc.declare_dram_parameter("beta", [1], f32, isOutput=False)
    colw = nc.declare_dram_parameter(
        "colw", [cfg.NCH * cfg.NGT, 128, GTS // 16], i16, isOutput=False)
    roww = nc.declare_dram_parameter(
        "roww", [cfg.NCH * cfg.NGT, 128, GTS // 16], i16, isOutput=False)
    riw = nc.declare_dram_parameter(
        "riw", [cfg.NCH * cfg.NGT, 128, GTS // 128], bf16, isOutput=False)
    brel = nc.declare_dram_parameter(
        "brel", [1, cfg.NW * cfg.NCH * cfg.SC], i32, isOutput=False)
    mult = nc.declare_dram_parameter("mult", [cfg.NSL, 1], f32, isOutput=False)
    out = nc.declare_dram_parameter("out", [cfg.NSL, D], f32, isOutput=True)

    gx_dram = nc.dram_tensor("gx_scratch", [cfg.NSL, D], f32)

    with ExitStack() as ctx:
        tc = ctx.enter_context(tile.TileContext(nc))
        consts = ctx.enter_context(tc.tile_pool(name="consts", bufs=1))
        keep = ctx.enter_context(tc.tile_pool(name="keep", bufs=1))

        # ---- constants -------------------------------------------------
        ident = consts.tile([128, 128], f32)
        from concourse.masks import make_identity
        make_identity(nc, ident[:])

        w_sb = consts.tile([D, D], f32)
        nc.sync.dma_start(w_sb[:], w[:, :])

        beta64 = consts.tile([D, 1], f32)
        nc.sync.dma_start(beta64[:], beta[:].to_broadcast((D, 1)))
        beta128 = consts.tile([128, 1], f32)
        nc.sync.dma_start(beta128[:], beta[:].to_broadcast((128, 1)))

        # w65: [W | 0; 0 | 1] so one matmul yields [h@W | s]
        w65 = consts.tile([D + 1, D + 1], f32)
        nc.vector.memset(w65[:], 0.0)
        nc.vector.tensor_copy(w65[:D, :D], w_sb[:])
        nc.vector.memset(w65[D:D + 1, D:D + 1], 1.0)

        # iota over the 64-node subtile window, broadcast along subtiles
        iota64 = consts.tile([128, 1, 64], bf16)
        nc.gpsimd.iota(iota64[:], pattern=[[1, 64]], base=0,
                       channel_multiplier=0,
                       allow_small_or_imprecise_dtypes=True)

        zrow1 = consts.tile([1, D + 1], bf16)
        nc.vector.memset(zrow1[:], 0.0)
        zrow512 = consts.tile([1, cfg.WIN], bf16)
        nc.vector.memset(zrow512[:], 0.0)

        brel_sb = consts.tile([1, cfg.NW * cfg.NCH * cfg.SC], i32)
        nc.sync.dma_start(brel_sb[:], brel[:, :])

        # persistent accumulators
        h_accT = keep.tile([D + 1, HACC_W], f32)
        nc.vector.memset(h_accT[:], 0.0)
        xp_all = keep.tile([128, cfg.NODE_TILES, D], f32)
        sattm_all = keep.tile([128, cfg.NODE_TILES, 1], f32)

        # ---- phase A: local node slice ----------------------------------
        with tc.tile_pool(name="sbA", bufs=3) as sb, \
             tc.tile_pool(name="psA", bufs=2, space="PSUM") as ps:

            # G = beta * W @ W^T (symmetric)
            wT_ps = ps.tile([D, D], f32, space="PSUM", tag="pA")
            nc.tensor.transpose(out=wT_ps[:], in_=w_sb[:],
                                identity=ident[:D, :D])
            wT_sb = consts.tile([D, D], f32)
            nc.vector.tensor_copy(wT_sb[:], wT_ps[:])
            g_ps = ps.tile([D, D], f32, space="PSUM", tag="pB")
            nc.tensor.matmul(out=g_ps[:], lhsT=wT_sb[:], rhs=wT_sb[:],
                             start=True, stop=True)
            gb_sb = consts.tile([D, D], f32)
            nc.scalar.mul(gb_sb[:], g_ps[:], beta64[:])

            for t in range(cfg.NODE_TILES):
                xt = sb.tile([128, D], f32, tag="xt")
                nc.sync.dma_start(xt[:], xsl[ts(t, 128), :])
                mult_t = sb.tile([128, 1], f32, tag="mult_t")
                nc.sync.dma_start(mult_t[:], mult[ts(t, 128), :])

                xtT_ps = ps.tile([D, 128], f32, space="PSUM", tag="pA")
                nc.tensor.transpose(out=xtT_ps[:], in_=xt[:],
                                    identity=ident[:])
                xtT_sb = sb.tile([D, 128], f32, tag="xtT_sb")
                nc.any.tensor_copy(xtT_sb[:], xtT_ps[:])

                # (G x)^T -> transpose -> node-major -> DRAM (gather source)
                gx_ps = ps.tile([D, 128], f32, space="PSUM", tag="pB")
                nc.tensor.matmul(out=gx_ps[:], lhsT=gb_sb[:], rhs=xtT_sb[:],
                                 start=True, stop=True)
                gx_sb = sb.tile([D, 128], f32, tag="gx_sb")
                nc.any.tensor_copy(gx_sb[:], gx_ps[:])
                gxT_ps = ps.tile([128, D], f32, space="PSUM", tag="pC")
                nc.tensor.transpose(out=gxT_ps[:], in_=gx_sb[:],
                                    identity=ident[:D, :D])
                gxn = sb.tile([128, D], f32, tag="gxn")
                nc.any.tensor_copy(gxn[:], gxT_ps[:])
                nc.sync.dma_start(gx_dram[ts(t, 128), :], gxn[:])

                # Xp^T -> node-major (kept in SBUF for self path + output)
                xpT_ps = ps.tile([D, 128], f32, space="PSUM", tag="pD")
                nc.tensor.matmul(out=xpT_ps[:], lhsT=w_sb[:], rhs=xtT_sb[:],
                                 start=True, stop=True)
                xpT_sb = sb.tile([D, 128], f32, tag="xpT_sb")
                nc.any.tensor_copy(xpT_sb[:], xpT_ps[:])
                xp_ps = ps.tile([128, D], f32, space="PSUM", tag="pC")
                nc.tensor.transpose(out=xp_ps[:], in_=xpT_sb[:],
                                    identity=ident[:D, :D])
                nc.vector.tensor_copy(xp_all[:, t, :], xp_ps[:])

                # self-loop attention (norm from f32 PSUM, not bf16 copy)
                sq = sb.tile([128, D], f32, tag="sq")
                nc.scalar.square(sq[:], xp_ps[:])
                nrm = sb.tile([128, 1], f32, tag="nrm")
                nc.vector.tensor_reduce(out=nrm[:], in_=sq[:],
                                        axis=mybir.AxisListType.X,
                                        op=Alu.add)
                satt = sb.tile([128, 1], f32, tag="satt")
                nc.scalar.activation(out=satt[:], in_=nrm[:], func=Act.Exp,
                                     scale=beta128[:])
                nc.vector.tensor_tensor(out=sattm_all[:, t, :], in0=satt[:],
                                        in1=mult_t[:], op=Alu.mult)

        # ---- phase B: edge stream ---------------------------------------
        with tc.tile_pool(name="sbB", bufs=2) as sbB, \
             tc.tile_pool(name="sbG", bufs=3) as sbG, \
             tc.tile_pool(name="psB", bufs=1, space="PSUM") as psB:

            for gt in range(cfg.NGT):
                hps = []
                for j in range(cfg.WPG):
                    hps_j = psB.tile([D + 1, cfg.WIN], f32, space="PSUM",
                                     tag=f"hw{j}", name=f"hps_{gt}_{j}")
                    hps.append(hps_j)
                dbg_nozero = bool(int(os.environ.get("AGNN_DBG_NOZERO", "0")))
                if not dbg_nozero:
                    for j in range(cfg.WPG):
                        nc.tensor.matmul(out=hps[j][:], lhsT=zrow1[:],
                                         rhs=zrow512[:], start=True, stop=False,
                                         skip_group_check=True)

                for c in range(cfg.NCH):
                    i = c * cfg.NGT + gt
                    colw_sb = sbB.tile([128, GTS // 16], i16, tag="colw")
                    nc.sync.dma_start(colw_sb[:], colw[i, :, :])
                    roww_sb = sbB.tile([128, GTS // 16], i16, tag="roww")
                    nc.sync.dma_start(roww_sb[:], roww[i, :, :])
                    riw_sb = sbB.tile([128, GTS // 128], bf16, tag="riw")
                    nc.sync.dma_start(riw_sb[:], riw[i, :, :])

                    HS = NSUB // 2
                    H1 = HS * 128
                    H2 = GTS - H1
                    xq = sbG.tile([128, NSUB, D], f32, tag="xq")
                    nc.gpsimd.dma_gather(
                        xq[:, :HS, :], xf[c * cfg.CHUNK:(c + 1) * cfg.CHUNK, :],
                        colw_sb[:, :H1 // 16], H1, H1, D, single_packet=False,
                        queue_num=0)
                    nc.gpsimd.dma_gather(
                        xq[:, HS:, :], xf[c * cfg.CHUNK:(c + 1) * cfg.CHUNK, :],
                        colw_sb[:, H1 // 16:], H2, H2, D, single_packet=False,
                        queue_num=1)
                    gxr = sbB.tile([128, NSUB, D], f32, tag="gxr")
                    nc.gpsimd.dma_gather(
                        gxr[:, :HS, :], gx_dram[:, :], roww_sb[:, :H1 // 16],
                        H1, H1, D, single_packet=False, queue_num=2)
                    nc.gpsimd.dma_gather(
                        gxr[:, HS:, :], gx_dram[:, :], roww_sb[:, H1 // 16:],
                        H2, H2, D, single_packet=False, queue_num=3)

                    # e = <x_col, G x_row>; att = exp(e)  (prod overwrites gxr)
                    nc.vector.tensor_tensor(out=gxr[:], in0=xq[:], in1=gxr[:],
                                            op=Alu.mult)
                    att = sbB.tile([128, NSUB, 1], f32, tag="att")
                    nc.vector.tensor_reduce(out=att[:], in_=gxr[:],
                                            axis=mybir.AxisListType.X,
                                            op=Alu.add)
                    nc.scalar.activation(out=att[:], in_=att[:], func=Act.Exp)

                    # payload = [att * x_col | att]  (bf16)
                    payload = sbB.tile([128, NSUB, D + 1], bf16, tag="payload")
                    nc.vector.tensor_tensor(
                        out=payload[:, :, 0:D], in0=xq[:],
                        in1=att[:].broadcast_to((128, NSUB, D)), op=Alu.mult)
                    nc.scalar.copy(payload[:, :, D:D + 1], att[:])

                    # one-hot mask [128e, 64w] per subtile
                    mask = sbB.tile([128, NSUB, 64], bf16, tag="mask")
                    nc.vector.tensor_tensor(
                        out=mask[:],
                        in0=iota64[:].broadcast_to((128, NSUB, 64)),
                        in1=riw_sb[:, :, None].broadcast_to((128, NSUB, 64)),
                        op=Alu.is_equal)

                    # aggregate: per window, per subtile
                    dbg_static = bool(int(os.environ.get("AGNN_DBG_STATIC", "0")))
                    for j in range(cfg.WPG):
                        woff = ((gt * cfg.WPG + j) * cfg.NCH + c) * cfg.SC
                        if dbg_static:
                            vals = [0] * cfg.SC
                        else:
                            _, vals = nc.values_load_multi_w_load_instructions(
                                brel_sb[0:1, woff:woff + cfg.SC],
                                engines=[mybir.EngineType.PE],
                                min_val=0, max_val=cfg.WIN - 64,
                                skip_runtime_bounds_check=True)
                        for k in range(cfg.SC):
                            s = j * cfg.SC + k
                            last = (c == cfg.NCH - 1) and (k == cfg.SC - 1)
                            nc.tensor.matmul(
                                out=hps[j][:, ds(vals[k], 64)],
                                lhsT=payload[:, s, :], rhs=mask[:, s, :],
                                start=False, stop=last,
                                skip_group_check=True)

                for j in range(cfg.WPG):
                    wb = (gt * cfg.WPG + j) * cfg.WIN
                    nc.vector.tensor_tensor(
                        out=h_accT[:, wb:wb + cfg.WIN],
                        in0=h_accT[:, wb:wb + cfg.WIN],
                        in1=hps[j][:], op=Alu.add)

        # ---- final: combine self path, project, divide -------------------
        with tc.tile_pool(name="sbF", bufs=3) as sbF, \
             tc.tile_pool(name="psF", bufs=2, space="PSUM") as psF:
            for t in range(cfg.NODE_TILES):
                proj_ps = psF.tile([128, D + 1], f32, space="PSUM", tag="pF")
                nc.tensor.matmul(out=proj_ps[:],
                                 lhsT=h_accT[:, ts(t, 128)], rhs=w65[:],
                                 start=True, stop=True)
                hfin = sbF.tile([128, D], f32, tag="hfin")
                nc.vector.scalar_tensor_tensor(
                    out=hfin[:], in0=xp_all[:, t, :], in1=proj_ps[:, 0:D],
                    scalar=sattm_all[:, t, :], op0=Alu.mult, op1=Alu.add)
                s_t = sbF.tile([128, 1], f32, tag="s_t")
                nc.vector.tensor_tensor(out=s_t[:], in0=proj_ps[:, D:D + 1],
                                        in1=sattm_all[:, t, :], op=Alu.add)
                rcp = sbF.tile([128, 1], f32, tag="rcp")
                nc.vector.reciprocal(rcp[:], s_t[:])
                outt = sbF.tile([128, D], f32, tag="outt")
                nc.scalar.mul(outt[:], hfin[:], rcp[:])
                nc.sync.dma_start(out[ts(t, 128), :], outt[:])

    nc.compile()
    return nc


# --------------------------------------------------------------------------
# host-side sharding / index prep
# --------------------------------------------------------------------------
def _pack_group(rows: np.ndarray, wb: int, win: int) -> list:
    """Greedy subtiles: <=128 edges each, rows within [b, b+64),
    b = min(first_row, wb + win - 64). Returns [(start, end, b), ...]."""
    subs = []
    i, n = 0, len(rows)
    while i < n:
        b = min(int(rows[i]), wb + win - 64)
        j = min(i + 128, n)
        j = min(j, int(np.searchsorted(rows, b + 64, side="left")))
        assert j > i
        subs.append((i, j, b))
        i = j
    return subs


def _required_sc(cfg: Cfg, row, col):
    """Max subtiles over all (core, window, chunk) groups."""
    row = np.asarray(row).astype(np.int64)
    col = np.asarray(col).astype(np.int64)
    mx = 0
    for i in range(cfg.CORES):
        lo = i * cfg.NSL_REAL
        sel = (row >= lo) & (row < lo + cfg.NSL_REAL)
        r = row[sel] - lo
        c = col[sel]
        ns = c != r + lo
        r, c = r[ns], c[ns]
        key = (r // cfg.WIN) * cfg.NCH + c // cfg.CHUNK
        order = np.argsort(key, kind="stable")
        r, key = r[order], key[order]
        bounds = np.searchsorted(key, np.arange(cfg.NW * cfg.NCH + 1))
        for g in range(cfg.NW * cfg.NCH):
            a, b = bounds[g], bounds[g + 1]
            if a == b:
                continue
            wb = (g // cfg.NCH) * cfg.WIN
            mx = max(mx, len(_pack_group(r[a:b], wb, cfg.WIN)))
    return mx


def _wrap16(a: np.ndarray) -> np.ndarray:
    """[T, GTS] -> [T, 128, GTS//16]: slot j -> [j%16, j//16], tiled x8."""
    t = a.shape[0]
    w = a.reshape(t, -1, 16).transpose(0, 2, 1)
    return np.ascontiguousarray(np.tile(w, (1, 8, 1)))


def _wrap128(a: np.ndarray) -> np.ndarray:
    """[T, GTS] -> [T, 128, GTS//128]: slot j -> [j%128, j//128]."""
    t = a.shape[0]
    return np.ascontiguousarray(a.reshape(t, -1, 128).transpose(0, 2, 1))


def prep_in_maps(cfg: Cfg, X, W, attention_w, row, col):
    X = np.ascontiguousarray(np.asarray(X, dtype=np.float32))
    W = np.ascontiguousarray(np.asarray(W, dtype=np.float32))
    beta = np.ascontiguousarray(np.asarray(attention_w, dtype=np.float32))
    row = np.asarray(row).astype(np.int64)
    col = np.asarray(col).astype(np.int64)

    xf = np.zeros((cfg.NPAD, cfg.D), dtype=np.float32)
    xf[:cfg.N] = X

    in_maps = []
    for i in range(cfg.CORES):
        lo = i * cfg.NSL_REAL
        hi = lo + cfg.NSL_REAL
        sel = (row >= lo) & (row < hi)
        r = row[sel] - lo
        c = col[sel]

        is_self = (c == r + lo)
        m = np.bincount(r[is_self], minlength=cfg.NSL).astype(np.float32)
        m[cfg.NSL_REAL:] = 1.0

        re = r[~is_self]
        ce = c[~is_self]
        key = (re // cfg.WIN) * cfg.NCH + ce // cfg.CHUNK
        order = np.argsort(key, kind="stable")
        re, ce, key = re[order], ce[order], key[order]
        bounds = np.searchsorted(key, np.arange(cfg.NW * cfg.NCH + 1))

        col16 = np.zeros((cfg.NCH, cfg.SPS), dtype=np.int16)
        row16 = np.zeros((cfg.NCH, cfg.SPS), dtype=np.int16)
        riwf = np.full((cfg.NCH, cfg.SPS), -1.0, dtype=np.float32)
        brel = np.zeros(cfg.NW * cfg.NCH * cfg.SC, dtype=np.int32)

        for wdx in range(cfg.NW):
            wb = wdx * cfg.WIN
            for cc in range(cfg.NCH):
                g = wdx * cfg.NCH + cc
                a, b = bounds[g], bounds[g + 1]
                rg, cg = re[a:b], ce[a:b]
                subs = _pack_group(rg, wb, cfg.WIN) if b > a else []
                assert len(subs) <= cfg.SC, \
                    f"SC overflow: {len(subs)} > {cfg.SC}"
                base = (wdx * cfg.SC) * 128  # slot base in stream cc
                for k, (s0, s1, bsub) in enumerate(subs):
                    sl = base + k * 128
                    n = s1 - s0
                    col16[cc, sl:sl + n] = (cg[s0:s1] - cc * cfg.CHUNK
                                            ).astype(np.int16)
                    row16[cc, sl:sl + n] = rg[s0:s1].astype(np.int16)
                    riwf[cc, sl:sl + n] = (rg[s0:s1] - bsub).astype(np.float32)
                    brel[(wdx * cfg.NCH + cc) * cfg.SC + k] = bsub - wb

        xsl = np.zeros((cfg.NSL, cfg.D), dtype=np.float32)
        xsl[:cfg.NSL_REAL] = X[lo:hi]

        ngt = cfg.NGT
        in_maps.append({
            "xf": xf,
            "xsl": xsl,
            "w": W,
            "beta": beta,
            "colw": _wrap16(col16.reshape(cfg.NCH * ngt, cfg.GTS)),
            "roww": _wrap16(row16.reshape(cfg.NCH * ngt, cfg.GTS)),
            "riw": _wrap128(riwf.reshape(cfg.NCH * ngt, cfg.GTS)
                            ).astype(np_bf16),
            "brel": brel.reshape(1, -1),
            "mult": m.reshape(-1, 1),
        })
    return in_maps


def assemble_out(cfg: Cfg, results) -> np.ndarray:
    parts = [np.asarray(results[i]["out"])[:cfg.NSL_REAL]
             for i in range(cfg.CORES)]
    return np.ascontiguousarray(np.concatenate(parts, axis=0))


# --------------------------------------------------------------------------
# entry point
# --------------------------------------------------------------------------
_NC_CACHE = {}
LAST_RESULT = None


def kernel(X, W, attention_w, row, col) -> np.ndarray:
    global LAST_RESULT
    from concourse.bass_utils import run_bass_kernel_spmd

    sc = _required_sc(CFG, row, col) + 1
    cfg = Cfg(SC=max(sc, CFG.SC))
    if cfg not in _NC_CACHE:
        _NC_CACHE[cfg] = build_nc(cfg)
    nc = _NC_CACHE[cfg]

    in_maps = prep_in_maps(cfg, X, W, attention_w, row, col)
    trace = bool(int(os.environ.get("AGNN_TRACE", "0")))
    res = run_bass_kernel_spmd(
        nc, in_maps, core_ids=list(range(cfg.CORES)), trace=trace)
    LAST_RESULT = res
    return assemble_out(cfg, res.results)


# revision 17
# speedup vs baseline: 4.1316x; 1.1783x over previous
"""AGNNConv on 8 Trainium2 NeuronCores (Bass/Tile).

Math (reference):
    Xp  = X @ W
    e   = beta * <Xp[row], Xp[col]>          per edge
    att = exp(e)
    h_n = sum_{e: row=n} att_e * Xp[col_e]
    s_n = sum_{e: row=n} att_e
    out = h / s

Key algebraic restructuring (W applied zero times per edge):
    e      = <x_col, G x_row>,  G = beta * W W^T   (raw feature space)
    h_raw  = sum att_e * x_col_e                   (raw feature space)
    out    = (h_raw @ W + att_self*m*Xp) / (s + att_self*m)
Self-loop edges (row == col) are peeled off the edge stream and handled
densely per node (att_self = exp(beta*|Xp_n|^2), multiplicity m).

Sharding: destination rows block-partitioned over 8 cores (row is sorted,
so each core owns a contiguous node slice and all edges targeting it --
no cross-core reduction).  Full X is replicated in each core's DRAM as a
kernel input; x_col rows are fetched with dma_gather (int16 indices force
4 column-chunk buckets); G x_row rows are computed locally in phase A and
fetched with a second dma_gather.

Aggregation (race-free, no DMA scatter): edges are sorted by
(512-node window of row, col-chunk); each 128-edge subtile spans <= 64
destination nodes, so a one-hot mask [128 edges, 64 nodes] (built on DVE
from per-edge "row minus subtile window base" values) turns the segment
sum into one PE matmul: H += payload^T @ mask, accumulated in a PSUM
window [65, 512] at a per-subtile dynamic column offset.  One DVE add per
512-node window folds PSUM into a persistent SBUF accumulator h_accT.
payload = [att * x_col | att] so the softmax denominator rides along as
feature 64.
"""

import os
from contextlib import ExitStack
from dataclasses import dataclass

import numpy as np

try:
    from ml_dtypes import bfloat16 as np_bf16
except ImportError:  # pragma: no cover
    np_bf16 = None


# --------------------------------------------------------------------------
# configuration
# --------------------------------------------------------------------------
@dataclass(frozen=True)
class Cfg:
    N: int = 100000        # total nodes
    D: int = 64            # feature dim
    CORES: int = 8
    NCH: int = 4           # column chunks (int16 gather index range)
    SC: int = 12           # subtiles per (window, chunk) group (data-driven)
    WIN: int = 512         # nodes per PSUM accumulation window

    @property
    def NSL_REAL(self):
        return self.N // self.CORES

    @property
    def NSL(self):
        return ((self.NSL_REAL + 1 + 127) // 128) * 128

    @property
    def CHUNK(self):
        c = (self.N + self.NCH - 1) // self.NCH
        c = ((c + 63) // 64) * 64
        assert c <= 32767
        return c

    @property
    def NPAD(self):
        return self.CHUNK * self.NCH

    @property
    def NW(self):
        return (self.NSL + self.WIN - 1) // self.WIN  # windows

    @property
    def WPG(self):
        return 5 if self.NW % 5 == 0 else 1  # windows per gather tile

    @property
    def NGT(self):
        return self.NW // self.WPG           # gather tiles per chunk stream

    @property
    def GTS(self):
        return self.WPG * self.SC * 128      # slots per gather tile

    @property
    def SPS(self):
        return self.NW * self.SC * 128       # slots per chunk stream

    @property
    def NODE_TILES(self):
        return self.NSL // 128


CFG = Cfg()


# --------------------------------------------------------------------------
# device graph
# --------------------------------------------------------------------------
def build_nc(cfg: Cfg):
    from concourse import bacc, mybir, tile
    from concourse.bass import ts, ds

    f32 = mybir.dt.float32
    bf16 = mybir.dt.bfloat16
    i16 = mybir.dt.int16
    i32 = mybir.dt.int32
    D = cfg.D
    Alu = mybir.AluOpType
    Act = mybir.ActivationFunctionType
    GTS = cfg.GTS
    NSUB = cfg.WPG * cfg.SC        # subtiles per gather tile
    HACC_W = cfg.NSL + cfg.WIN     # h_accT column pad so ds(wb,512) stays in range

    nc = bacc.Bacc(
        "TRN2", target_bir_lowering=False, debug=False,
        num_devices=cfg.CORES, num_swdge_queues=4,
    )

    xf = nc.declare_dram_parameter("xf", [cfg.NPAD, D], f32, isOutput=False)
    xsl = nc.declare_dram_parameter("xsl", [cfg.NSL, D], f32, isOutput=False)
    w = nc.declare_dram_parameter("w", [D, D], f32, isOutput=False)
    beta = nc.declare_dram_parameter("beta", [1], f32, isOutput=False)
    colw = nc.declare_dram_parameter(
        "colw", [cfg.NCH * cfg.NGT, 128, GTS // 16], i16, isOutput=False)
    roww = nc.declare_dram_parameter(
        "roww", [cfg.NCH * cfg.NGT, 128, GTS // 16], i16, isOutput=False)
    riw = nc.declare_dram_parameter(
        "riw", [cfg.NCH * cfg.NGT, 128, GTS // 128], bf16, isOutput=False)
    brel = nc.declare_dram_parameter(
        "brel", [1, cfg.NW * cfg.NCH * cfg.SC], i32, isOutput=False)
    mult = nc.declare_dram_parameter("mult", [cfg.NSL, 1], f32, isOutput=False)
    out = nc.declare_dram_parameter("out", [cfg.NSL, D], f32, isOutput=True)

    gx_dram = nc.dram_tensor("gx_scratch", [cfg.NSL, D], f32)

    with ExitStack() as ctx:
        tc = ctx.enter_context(tile.TileContext(nc))
        consts = ctx.enter_context(tc.tile_pool(name="consts", bufs=1))
        keep = ctx.enter_context(tc.tile_pool(name="keep", bufs=1))

        # ---- constants -------------------------------------------------
        ident = consts.tile([128, 128], f32)
        from concourse.masks import make_identity
        make_identity(nc, ident[:])

        w_sb = consts.tile([D, D], f32)
        nc.sync.dma_start(w_sb[:], w[:, :])

        beta64 = consts.tile([D, 1], f32)
        nc.sync.dma_start(beta64[:], beta[:].to_broadcast((D, 1)))
        beta128 = consts.tile([128, 1], f32)
        nc.sync.dma_start(beta128[:], beta[:].to_broadcast((128, 1)))

        # w65: [W | 0; 0 | 1] so one matmul yields [h@W | s]
        w65 = consts.tile([D + 1, D + 1], f32)
        nc.vector.memset(w65[:], 0.0)
        nc.vector.tensor_copy(w65[:D, :D], w_sb[:])
        nc.vector.memset(w65[D:D + 1, D:D + 1], 1.0)

        # iota over the 64-node subtile window, broadcast along subtiles
        iota64 = consts.tile([128, 1, 64], bf16)
        nc.gpsimd.iota(iota64[:], pattern=[[1, 64]], base=0,
                       channel_multiplier=0,
                       allow_small_or_imprecise_dtypes=True)

        zrow1 = consts.tile([1, D + 1], bf16)
        nc.vector.memset(zrow1[:], 0.0)
        zrow512 = consts.tile([1, cfg.WIN], bf16)
        nc.vector.memset(zrow512[:], 0.0)

        brel_sb = consts.tile([1, cfg.NW * cfg.NCH * cfg.SC], i32)
        nc.sync.dma_start(brel_sb[:], brel[:, :])

        # persistent accumulators
        h_accT = keep.tile([D + 1, HACC_W], f32)
        nc.vector.memset(h_accT[:], 0.0)
        xp_all = keep.tile([128, cfg.NODE_TILES, D], f32)
        sattm_all = keep.tile([128, cfg.NODE_TILES, 1], f32)

        # ---- phase A: local node slice ----------------------------------
        with tc.tile_pool(name="sbA", bufs=3) as sb, \
             tc.tile_pool(name="psA", bufs=2, space="PSUM") as ps:

            # G = beta * W @ W^T (symmetric)
            wT_ps = ps.tile([D, D], f32, space="PSUM", tag="pA")
            nc.tensor.transpose(out=wT_ps[:], in_=w_sb[:],
                                identity=ident[:D, :D])
            wT_sb = consts.tile([D, D], f32)
            nc.vector.tensor_copy(wT_sb[:], wT_ps[:])
            g_ps = ps.tile([D, D], f32, space="PSUM", tag="pB")
            nc.tensor.matmul(out=g_ps[:], lhsT=wT_sb[:], rhs=wT_sb[:],
                             start=True, stop=True)
            gb_sb = consts.tile([D, D], f32)
            nc.scalar.mul(gb_sb[:], g_ps[:], beta64[:])

            for t in range(cfg.NODE_TILES):
                xt = sb.tile([128, D], f32, tag="xt")
                nc.sync.dma_start(xt[:], xsl[ts(t, 128), :])
                mult_t = sb.tile([128, 1], f32, tag="mult_t")
                nc.sync.dma_start(mult_t[:], mult[ts(t, 128), :])

                xtT_ps = ps.tile([D, 128], f32, space="PSUM", tag="pA")
                nc.tensor.transpose(out=xtT_ps[:], in_=xt[:],
                                    identity=ident[:])
                xtT_sb = sb.tile([D, 128], f32, tag="xtT_sb")
                nc.any.tensor_copy(xtT_sb[:], xtT_ps[:])

                # (G x)^T -> transpose -> node-major -> DRAM (gather source)
                gx_ps = ps.tile([D, 128], f32, space="PSUM", tag="pB")
                nc.tensor.matmul(out=gx_ps[:], lhsT=gb_sb[:], rhs=xtT_sb[:],
                                 start=True, stop=True)
                gx_sb = sb.tile([D, 128], f32, tag="gx_sb")
                nc.any.tensor_copy(gx_sb[:], gx_ps[:])
                gxT_ps = ps.tile([128, D], f32, space="PSUM", tag="pC")
                nc.tensor.transpose(out=gxT_ps[:], in_=gx_sb[:],
                                    identity=ident[:D, :D])
                gxn = sb.tile([128, D], f32, tag="gxn")
                nc.any.tensor_copy(gxn[:], gxT_ps[:])
                nc.sync.dma_start(gx_dram[ts(t, 128), :], gxn[:])

                # Xp^T -> node-major (kept in SBUF for self path + output)
                xpT_ps = ps.tile([D, 128], f32, space="PSUM", tag="pD")
                nc.tensor.matmul(out=xpT_ps[:], lhsT=w_sb[:], rhs=xtT_sb[:],
                                 start=True, stop=True)
                xpT_sb = sb.tile([D, 128], f32, tag="xpT_sb")
                nc.any.tensor_copy(xpT_sb[:], xpT_ps[:])
                xp_ps = ps.tile([128, D], f32, space="PSUM", tag="pC")
                nc.tensor.transpose(out=xp_ps[:], in_=xpT_sb[:],
                                    identity=ident[:D, :D])
                nc.vector.tensor_copy(xp_all[:, t, :], xp_ps[:])

                # self-loop attention (norm from f32 PSUM, not bf16 copy)
                sq = sb.tile([128, D], f32, tag="sq")
                nc.scalar.square(sq[:], xp_ps[:])
                nrm = sb.tile([128, 1], f32, tag="nrm")
                nc.vector.tensor_reduce(out=nrm[:], in_=sq[:],
                                        axis=mybir.AxisListType.X,
                                        op=Alu.add)
                satt = sb.tile([128, 1], f32, tag="satt")
                nc.scalar.activation(out=satt[:], in_=nrm[:], func=Act.Exp,
                                     scale=beta128[:])
                nc.vector.tensor_tensor(out=sattm_all[:, t, :], in0=satt[:],
                                        in1=mult_t[:], op=Alu.mult)

        # ---- phase B: edge stream ---------------------------------------
        with tc.tile_pool(name="sbB", bufs=2) as sbB, \
             tc.tile_pool(name="sbG", bufs=3) as sbG, \
             tc.tile_pool(name="psB", bufs=1, space="PSUM") as psB:

            for gt in range(cfg.NGT):
                hps = []
                for j in range(cfg.WPG):
                    hps_j = psB.tile([D + 1, cfg.WIN], f32, space="PSUM",
                                     tag=f"hw{j}", name=f"hps_{gt}_{j}")
                    hps.append(hps_j)
                dbg_nozero = bool(int(os.environ.get("AGNN_DBG_NOZERO", "0")))
                if not dbg_nozero:
                    for j in range(cfg.WPG):
                        nc.tensor.matmul(out=hps[j][:], lhsT=zrow1[:],
                                         rhs=zrow512[:], start=True, stop=False,
                                         skip_group_check=True)

                for c in range(cfg.NCH):
                    i = c * cfg.NGT + gt
                    colw_sb = sbB.tile([128, GTS // 16], i16, tag="colw")
                    nc.sync.dma_start(colw_sb[:], colw[i, :, :])
                    roww_sb = sbB.tile([128, GTS // 16], i16, tag="roww")
                    nc.sync.dma_start(roww_sb[:], roww[i, :, :])
                    riw_sb = sbB.tile([128, GTS // 128], bf16, tag="riw")
                    nc.sync.dma_start(riw_sb[:], riw[i, :, :])

                    HS = NSUB // 2
                    H1 = HS * 128
                    H2 = GTS - H1
                    xq = sbG.tile([128, NSUB, D], f32, tag="xq")
                    nc.gpsimd.dma_gather(
                        xq[:, :HS, :], xf[c * cfg.CHUNK:(c + 1) * cfg.CHUNK, :],
                        colw_sb[:, :H1 // 16], H1, H1, D, single_packet=False,
                        queue_num=0)
                    nc.gpsimd.dma_gather(
                        xq[:, HS:, :], xf[c * cfg.CHUNK:(c + 1) * cfg.CHUNK, :],
                        colw_sb[:, H1 // 16:], H2, H2, D, single_packet=False,
                        queue_num=1)
                    gxr = sbB.tile([128, NSUB, D], f32, tag="gxr")
                    nc.gpsimd.dma_gather(
                        gxr[:, :HS, :], gx_dram[:, :], roww_sb[:, :H1 // 16],
                        H1, H1, D, single_packet=False, queue_num=2)
                    nc.gpsimd.dma_gather(
                        gxr[:, HS:, :], gx_dram[:, :], roww_sb[:, H1 // 16:],
                        H2, H2, D, single_packet=False, queue_num=3)

                    # e = <x_col, G x_row>; att = exp(e)  (prod overwrites gxr)
                    nc.vector.tensor_tensor(out=gxr[:], in0=xq[:], in1=gxr[:],
                                            op=Alu.mult)
                    att = sbB.tile([128, NSUB, 1], f32, tag="att")
                    nc.vector.tensor_reduce(out=att[:], in_=gxr[:],
                                            axis=mybir.AxisListType.X,
                                            op=Alu.add)
                    nc.scalar.activation(out=att[:], in_=att[:], func=Act.Exp)

                    # payload = [att * x_col | att]  (bf16)
                    payload = sbB.tile([128, NSUB, D + 1], bf16, tag="payload")
                    nc.vector.tensor_tensor(
                        out=payload[:, :, 0:D], in0=xq[:],
                        in1=att[:].broadcast_to((128, NSUB, D)), op=Alu.mult)
                    nc.scalar.copy(payload[:, :, D:D + 1], att[:])

                    # one-hot mask [128e, 64w] per subtile
                    mask = sbB.tile([128, NSUB, 64], bf16, tag="mask")
                    nc.vector.tensor_tensor(
                        out=mask[:],
                        in0=iota64[:].broadcast_to((128, NSUB, 64)),
                        in1=riw_sb[:, :, None].broadcast_to((128, NSUB, 64)),
                        op=Alu.is_equal)

                    # aggregate: per window, per subtile
                    dbg_static = bool(int(os.environ.get("AGNN_DBG_STATIC", "0")))
                    for j in range(cfg.WPG):
                        woff = ((gt * cfg.WPG + j) * cfg.NCH + c) * cfg.SC
                        if dbg_static:
                            vals = [0] * cfg.SC
                        else:
                            _, vals = nc.values_load_multi_w_load_instructions(
                                brel_sb[0:1, woff:woff + cfg.SC],
                                engines=[mybir.EngineType.PE],
                                min_val=0, max_val=cfg.WIN - 64,
                                skip_runtime_bounds_check=True)
                        for k in range(cfg.SC):
                            s = j * cfg.SC + k
                            last = (c == cfg.NCH - 1) and (k == cfg.SC - 1)
                            nc.tensor.matmul(
                                out=hps[j][:, ds(vals[k], 64)],
                                lhsT=payload[:, s, :], rhs=mask[:, s, :],
                                start=False, stop=last,
                                skip_group_check=True)

                for j in range(cfg.WPG):
                    wb = (gt * cfg.WPG + j) * cfg.WIN
                    nc.vector.tensor_tensor(
                        out=h_accT[:, wb:wb + cfg.WIN],
                        in0=h_accT[:, wb:wb + cfg.WIN],
                        in1=hps[j][:], op=Alu.add)

        # ---- final: combine self path, project, divide -------------------
        with tc.tile_pool(name="sbF", bufs=3) as sbF, \
             tc.tile_pool(name="psF", bufs=2, space="PSUM") as psF:
            for t in range(cfg.NODE_TILES):
                proj_ps = psF.tile([128, D + 1], f32, space="PSUM", tag="pF")
                nc.tensor.matmul(out=proj_ps[:],
                                 lhsT=h_accT[:, ts(t, 128)], rhs=w65[:],
                                 start=True, stop=True)
                hfin = sbF.tile([128, D], f32, tag="hfin")
                nc.vector.scalar_tensor_tensor(
                    out=hfin[:], in0=xp_all[:, t, :], in1=proj_ps[:, 0:D],
                    scalar=sattm_all[:, t, :], op0=Alu.mult, op1=Alu.add)
                s_t = sbF.tile([128, 1], f32, tag="s_t")
                nc.vector.tensor_tensor(out=s_t[:], in0=proj_ps[:, D:D + 1],
                                        in1=sattm_all[:, t, :], op=Alu.add)
                rcp = sbF.tile([128, 1], f32, tag="rcp")
                nc.vector.reciprocal(rcp[:], s_t[:])
                outt = sbF.tile([128, D], f32, tag="outt")
                nc.scalar.mul(outt[:], hfin[:], rcp[:])
                nc.sync.dma_start(out[ts(t, 128), :], outt[:])

    nc.compile()
    return nc


# --------------------------------------------------------------------------
# host-side sharding / index prep
# --------------------------------------------------------------------------
def _pack_group(rows: np.ndarray, wb: int, win: int) -> list:
    """Greedy subtiles: <=128 edges each, rows within [b, b+64),
    b = min(first_row, wb + win - 64). Returns [(start, end, b), ...]."""
    subs = []
    i, n = 0, len(rows)
    while i < n:
        b = min(int(rows[i]), wb + win - 64)
        j = min(i + 128, n)
        j = min(j, int(np.searchsorted(rows, b + 64, side="left")))
        assert j > i
        subs.append((i, j, b))
        i = j
    return subs


def _required_sc(cfg: Cfg, row, col):
    """Max subtiles over all (core, window, chunk) groups."""
    row = np.asarray(row).astype(np.int64)
    col = np.asarray(col).astype(np.int64)
    mx = 0
    for i in range(cfg.CORES):
        lo = i * cfg.NSL_REAL
        sel = (row >= lo) & (row < lo + cfg.NSL_REAL)
        r = row[sel] - lo
        c = col[sel]
        ns = c != r + lo
        r, c = r[ns], c[ns]
        key = (r // cfg.WIN) * cfg.NCH + c // cfg.CHUNK
        order = np.argsort(key, kind="stable")
        r, key = r[order], key[order]
        bounds = np.searchsorted(key, np.arange(cfg.NW * cfg.NCH + 1))
        for g in range(cfg.NW * cfg.NCH):
            a, b = bounds[g], bounds[g + 1]
            if a == b:
                continue
            wb = (g // cfg.NCH) * cfg.WIN
            mx = max(mx, len(_pack_group(r[a:b], wb, cfg.WIN)))
    return mx


def _wrap16(a: np.ndarray) -> np.ndarray:
    """[T, GTS] -> [T, 128, GTS//16]: slot j -> [j%16, j//16], tiled x8."""
    t = a.shape[0]
    w = a.reshape(t, -1, 16).transpose(0, 2, 1)
    return np.ascontiguousarray(np.tile(w, (1, 8, 1)))


def _wrap128(a: np.ndarray) -> np.ndarray:
    """[T, GTS] -> [T, 128, GTS//128]: slot j -> [j%128, j//128]."""
    t = a.shape[0]
    return np.ascontiguousarray(a.reshape(t, -1, 128).transpose(0, 2, 1))


def prep_in_maps(cfg: Cfg, X, W, attention_w, row, col):
    X = np.ascontiguousarray(np.asarray(X, dtype=np.float32))
    W = np.ascontiguousarray(np.asarray(W, dtype=np.float32))
    beta = np.ascontiguousarray(np.asarray(attention_w, dtype=np.float32))
    row = np.asarray(row).astype(np.int64)
    col = np.asarray(col).astype(np.int64)

    xf = np.zeros((cfg.NPAD, cfg.D), dtype=np.float32)
    xf[:cfg.N] = X

    in_maps = []
    for i in range(cfg.CORES):
        lo = i * cfg.NSL_REAL
        hi = lo + cfg.NSL_REAL
        sel = (row >= lo) & (row < hi)
        r = row[sel] - lo
        c = col[sel]

        is_self = (c == r + lo)
        m = np.bincount(r[is_self], minlength=cfg.NSL).astype(np.float32)
        m[cfg.NSL_REAL:] = 1.0

        re = r[~is_self]
        ce = c[~is_self]
        key = (re // cfg.WIN) * cfg.NCH + ce // cfg.CHUNK
        order = np.argsort(key, kind="stable")
        re, ce, key = re[order], ce[order], key[order]
        bounds = np.searchsorted(key, np.arange(cfg.NW * cfg.NCH + 1))

        col16 = np.zeros((cfg.NCH, cfg.SPS), dtype=np.int16)
        row16 = np.zeros((cfg.NCH, cfg.SPS), dtype=np.int16)
        riwf = np.full((cfg.NCH, cfg.SPS), -1.0, dtype=np.float32)
        brel = np.zeros(cfg.NW * cfg.NCH * cfg.SC, dtype=np.int32)

        for wdx in range(cfg.NW):
            wb = wdx * cfg.WIN
            for cc in range(cfg.NCH):
                g = wdx * cfg.NCH + cc
                a, b = bounds[g], bounds[g + 1]
                rg, cg = re[a:b], ce[a:b]
                subs = _pack_group(rg, wb, cfg.WIN) if b > a else []
                assert len(subs) <= cfg.SC, \
                    f"SC overflow: {len(subs)} > {cfg.SC}"
                base = (wdx * cfg.SC) * 128  # slot base in stream cc
                for k, (s0, s1, bsub) in enumerate(subs):
                    sl = base + k * 128
                    n = s1 - s0
                    col16[cc, sl:sl + n] = (cg[s0:s1] - cc * cfg.CHUNK
                                            ).astype(np.int16)
                    row16[cc, sl:sl + n] = rg[s0:s1].astype(np.int16)
                    riwf[cc, sl:sl + n] = (rg[s0:s1] - bsub).astype(np.float32)
                    brel[(wdx * cfg.NCH + cc) * cfg.SC + k] = bsub - wb

        xsl = np.zeros((cfg.NSL, cfg.D), dtype=np.float32)
        xsl[:cfg.NSL_REAL] = X[lo:hi]

        ngt = cfg.NGT
        in_maps.append({
            "xf": xf,
            "xsl": xsl,
            "w": W,
            "beta": beta,
            "colw": _wrap16(col16.reshape(cfg.NCH * ngt, cfg.GTS)),
            "roww": _wrap16(row16.reshape(cfg.NCH * ngt, cfg.GTS)),
            "riw": _wrap128(riwf.reshape(cfg.NCH * ngt, cfg.GTS)
                            ).astype(np_bf16),
            "brel": brel.reshape(1, -1),
            "mult": m.reshape(-1, 1),
        })
    return in_maps


def assemble_out(cfg: Cfg, results) -> np.ndarray:
    parts = [np.asarray(results[i]["out"])[:cfg.NSL_REAL]
             for i in range(cfg.CORES)]
    return np.ascontiguousarray(np.concatenate(parts, axis=0))


# --------------------------------------------------------------------------
# entry point
# --------------------------------------------------------------------------
_NC_CACHE = {}
LAST_RESULT = None


def kernel(X, W, attention_w, row, col) -> np.ndarray:
    global LAST_RESULT
    from concourse.bass_utils import run_bass_kernel_spmd

    sc = _required_sc(CFG, row, col)
    cfg = Cfg(SC=max(sc, 2))
    if cfg not in _NC_CACHE:
        _NC_CACHE[cfg] = build_nc(cfg)
    nc = _NC_CACHE[cfg]

    in_maps = prep_in_maps(cfg, X, W, attention_w, row, col)
    trace = bool(int(os.environ.get("AGNN_TRACE", "0")))
    res = run_bass_kernel_spmd(
        nc, in_maps, core_ids=list(range(cfg.CORES)), trace=trace)
    LAST_RESULT = res
    return assemble_out(cfg, res.results)
